# revision 12
# baseline (speedup 1.0000x reference)
"""GATv2 attention-pool kernel for 8 Trainium2 NeuronCores.

v2 "diagonal scatter" design
----------------------------
Reference computes, per edge e with target node t(e):
    feats = q + k                                   [E, 64]
    logits[e,h] = sum_c feats[e,h*8+c] * A[c,h]     [E, 8]
    attn = segment_softmax(logits, targets)         [E, 8]
    out[n] = relu(segment_sum(q * attn))            [N, 64]

Logits are O(10) so exp() never overflows fp32; the segment-max shift is
unnecessary and softmax folds into two segment-SUMS:
    denom[n,h]  = sum_{e->n} exp(logits[e,h])
    pooled[n,:] = sum_{e->n} q[e,:] * exp(logits[e,h])
    out[n]      = relu(pooled[n]) / denom[n]

Layout: nodes are sorted by degree and dealt round-robin to the 8 cores so
every core sees an identical degree profile (one SPMD program).  A window is
128 similar-degree nodes, ONE NODE PER SBUF PARTITION; each node's edge list
is padded to the window max Tw (~5% padding, pad rows are crafted so their
logits are ~-30 for every head -> exp ~ 1e-13 contributes nothing).  With
that layout the segment reductions are plain free-dim sums per partition:
    psum[128, 72] += I128^T @ m[:, t, 0:72]     (t = 0..Tw-1)
i.e. PSUM accumulation through the PE with a CONSTANT identity stationary
matrix - no per-subtile one-hot selector, no index tensors on device at all.
Everything ships and computes in bf16 (fp32 only for logits tail + PSUM).

Host work is index metadata + data layout only (degree sort, gather of q/k
rows into the padded slot order, bf16 cast); all floating-point math runs on
device.
"""

import os
import sys

import numpy as np

N_NODES = 100000
N_EDGES = 1600000
H = 8
C = 8
HC = H * C
N_CORES = 8
P = 128                       # nodes per window == SBUF partitions
NPC = N_NODES // N_CORES      # nodes per core
N_WIN = (NPC + P - 1) // P    # windows per core
TCAP = 80                     # max t-columns per processing group
NWCAP = 7                     # psum bank holds nw*72 fp32 <= 512
PAD_LG = -30.0                # logit forced onto pad slots
F_VIA_DMA_ACCUM = False       # build f=q+k with an accumulating SWDGE DMA


def _ensure_imports():
    try:
        import concourse.bass  # noqa: F401
    except ImportError:
        for p in ("/opt/trn_rl_repo", "/root/.axon_site/_ro/trn_rl_repo"):
            if os.path.isdir(p) and p not in sys.path:
                sys.path.insert(0, p)


def _run_arange(counts):
    """[0..c0-1, 0..c1-1, ...] for run lengths `counts`."""
    tot = int(counts.sum())
    a = np.arange(tot, dtype=np.int64)
    off = np.repeat(np.cumsum(counts) - counts, counts)
    return a - off


def preprocess(targets):
    """Degree-sort nodes, deal round-robin to cores, pack windows.

    Returns (order, deg, Tw [N_WIN], c0 [N_WIN+1], groups, C_total,
    idx_maps: per-core [P, C_total] int64 edge id or -1 for pad).
    """
    deg = np.bincount(targets, minlength=N_NODES).astype(np.int64)
    order = np.argsort(deg, kind="stable")          # ascending degree
    Tw = np.zeros(N_WIN, dtype=np.int64)
    for w in range(N_WIN):
        lo = w * P * N_CORES
        hi = min((w + 1) * P * N_CORES, N_NODES)
        Tw[w] = max(1, int(deg[order[lo:hi]].max()))
    c0 = np.zeros(N_WIN + 1, dtype=np.int64)
    c0[1:] = np.cumsum(Tw)
    C_total = int(c0[-1])

    groups = []  # (w0, nw, T, cstart)
    w = 0
    while w < N_WIN:
        w0 = w
        T = 0
        while w < N_WIN and (w - w0) < NWCAP and T + Tw[w] <= TCAP:
            T += int(Tw[w])
            w += 1
        if w == w0:          # single window wider than TCAP
            T = int(Tw[w])
            w += 1
        groups.append((w0, w - w0, T, int(c0[w0])))

    eorder = np.argsort(targets, kind="stable")
    tsorted = targets[eorder]
    estart = np.searchsorted(tsorted, np.arange(N_NODES + 1))

    idx_maps = []
    for cidx in range(N_CORES):
        nodes = order[cidx::N_CORES]                # local rank l -> node
        l = np.arange(len(nodes), dtype=np.int64)
        wloc = l // P
        ploc = l % P
        d = deg[nodes]
        dstart = ploc * C_total + c0[wloc]
        dst = np.repeat(dstart, d) + _run_arange(d)
        src = np.repeat(estart[nodes], d) + _run_arange(d)
        idx = np.full(P * C_total, -1, dtype=np.int64)
        idx[dst] = eorder[src]
        idx_maps.append(idx.reshape(P, C_total))
    return order, deg, Tw, c0, groups, C_total, idx_maps


def build_nc(groups, Tw, C_total):
    """Single SPMD Bass program for one core's shard."""
    _ensure_imports()
    import concourse.bacc as bacc
    import concourse.mybir as mybir
    import concourse.tile as tile

    f32 = mybir.dt.float32
    bf16 = mybir.dt.bfloat16
    f16 = mybir.dt.float16
    AF = mybir.ActivationFunctionType
    OP = mybir.AluOpType

    # fp16 for the small-range logits path (8x finer mantissa than bf16);
    # bf16 only where exp() range demands it (eb, m); fp32 logits + PSUM.
    nc = bacc.Bacc("TRN2", num_devices=N_CORES)
    qd = nc.declare_dram_parameter("q", [P, C_total * HC], f16, False)
    kd = nc.declare_dram_parameter("k", [P, C_total * HC], f16, False)
    wd = nc.declare_dram_parameter("w", [P, TCAP * HC], f16, False)
    idd = nc.declare_dram_parameter("ident", [P, P], bf16, False)
    outd = nc.declare_dram_parameter("out", [P, N_WIN * HC], f16,
                                     isOutput=True)

    with tile.TileContext(nc) as tc:
        with (
            tc.tile_pool(name="const", bufs=1) as cpool,
            tc.tile_pool(name="qin", bufs=3) as qpool,
            tc.tile_pool(name="kin", bufs=3) as kpool,
            tc.tile_pool(name="mid", bufs=2) as midpool,
            tc.tile_pool(name="mm", bufs=2) as mpool,
            tc.tile_pool(name="fin", bufs=3) as finpool,
            tc.tile_pool(name="psum", bufs=8, space="PSUM") as ppool,
        ):
            w_t = cpool.tile([P, TCAP * HC], f16)
            nc.sync.dma_start(out=w_t[:], in_=wd[:])
            id_t = cpool.tile([P, P], bf16)
            nc.sync.dma_start(out=id_t[:], in_=idd[:])

            for (w0, nw, T, cs) in groups:
                fd = T * HC
                qt = qpool.tile([P, fd], f16, tag="q")
                nc.sync.dma_start(out=qt[:], in_=qd[:, cs * HC:(cs + T) * HC])

                if F_VIA_DMA_ACCUM:
                    # f = q + k built by the DMA engines: load q a second
                    # time, then the SWDGE k-load accumulates on top.
                    ft = kpool.tile([P, fd], f16, tag="f")
                    nc.sync.dma_start(out=ft[:],
                                      in_=qd[:, cs * HC:(cs + T) * HC])
                    nc.gpsimd.dma_start(out=ft[:],
                                        in_=kd[:, cs * HC:(cs + T) * HC],
                                        accum_op=OP.add)
                else:
                    kt = kpool.tile([P, fd], f16, tag="k")
                    nc.sync.dma_start(out=kt[:],
                                      in_=kd[:, cs * HC:(cs + T) * HC])
                    ft = midpool.tile([P, fd], f16, tag="f")
                    nc.vector.tensor_add(ft[:], qt[:], kt[:])
                wf = midpool.tile([P, fd], f16, tag="wf")
                nc.vector.tensor_mul(wf[:], ft[:], w_t[:, :fd])

                # tree-reduce the 8 channels per (t, head)
                wf4 = wf[:].rearrange("p (x c) -> p x c", c=8)
                t1 = midpool.tile([P, T * 8, 4], f16, tag="t1")
                nc.vector.tensor_add(t1[:], wf4[:, :, 0:4], wf4[:, :, 4:8])
                t2 = midpool.tile([P, T * 8, 2], f16, tag="t2")
                nc.vector.tensor_add(t2[:], t1[:, :, 0:2], t1[:, :, 2:4])
                lg = midpool.tile([P, T * 8], f32, tag="lg")
                nc.vector.tensor_add(lg[:], t2[:, :, 0], t2[:, :, 1])

                lg3 = lg[:].rearrange("p (t h) -> p t h", h=H)
                eb = midpool.tile([P, fd], bf16, tag="eb")
                nc.scalar.activation(
                    out=eb[:].rearrange("p (t h c) -> p t h c", h=H, c=C),
                    in_=lg3[:, :, :, None].to_broadcast([P, T, H, C]),
                    func=AF.Exp,
                )

                m = mpool.tile([P, T, 72], bf16, tag="m")
                nc.vector.tensor_mul(
                    m[:, :, 0:HC],
                    qt[:].rearrange("p (t j) -> p t j", j=HC),
                    eb[:].rearrange("p (t j) -> p t j", j=HC))
                nc.scalar.activation(out=m[:, :, HC:72], in_=lg3, func=AF.Exp)

                ps = ppool.tile([P, nw * 72], f32)
                tg = 0
                for wi in range(nw):
                    tw = int(Tw[w0 + wi])
                    for t in range(tw):
                        nc.tensor.matmul(
                            ps[:, wi * 72:(wi + 1) * 72],
                            lhsT=id_t[:],
                            rhs=m[:, tg + t, :],
                            start=(t == 0),
                            stop=(t == tw - 1),
                        )
                    tg += tw

                ps3 = ps[:].rearrange("p (w j) -> p w j", j=72)
                o = finpool.tile([P, nw, HC], f32, tag="o")
                nc.scalar.activation(out=o[:], in_=ps3[:, :, 0:HC],
                                     func=AF.Relu)
                rc = finpool.tile([P, nw, H], f32, tag="rc")
                nc.vector.reciprocal_approx_fast(rc[:], ps3[:, :, HC:72])
                o2 = finpool.tile([P, nw, HC], f16, tag="o2")
                nc.vector.tensor_mul(
                    o2[:].rearrange("p w (h c) -> p w h c", h=H),
                    o[:].rearrange("p w (h c) -> p w h c", h=H),
                    rc[:, :, :, None].to_broadcast([P, nw, H, C]),
                )
                nc.sync.dma_start(
                    out=outd[:, w0 * HC:(w0 + nw) * HC],
                    in_=o2[:].rearrange("p w j -> p (w j)"),
                )

    nc.finalize()
    return nc


def _host_arrays(query, key, attn_kernel, targets):
    import ml_dtypes

    bf = ml_dtypes.bfloat16
    f16 = np.float16
    order, deg, Tw, c0, groups, C_total, idx_maps = preprocess(targets)

    qb = query.astype(f16)
    kb = key.astype(f16)

    # pad rows: q = 0; k chosen so logits[h] == PAD_LG for every head
    A = attn_kernel.astype(np.float64)               # [C, H]
    nrm = np.maximum((A * A).sum(axis=0), 1e-6)      # ||A[:,h]||^2
    v = (PAD_LG / nrm)[None, :] * A                  # [C, H]
    kpad_row = np.ascontiguousarray(v.T).reshape(-1).astype(f16)  # [h*8+c]

    wrow = np.ascontiguousarray(attn_kernel.T).reshape(-1)  # [h*8+c] = A[c,h]
    w_arr = np.tile(wrow, (P, TCAP)).astype(f16)
    ident = np.eye(P, dtype=np.float32).astype(bf)

    in_maps = []
    for cidx in range(N_CORES):
        idx = idx_maps[cidx]
        safe = np.maximum(idx, 0)
        pad = idx < 0
        qdev = qb[safe]
        qdev[pad] = 0
        kdev = kb[safe]
        kdev[pad] = kpad_row
        in_maps.append({
            "q": np.ascontiguousarray(qdev.reshape(P, C_total * HC)),
            "k": np.ascontiguousarray(kdev.reshape(P, C_total * HC)),
            "w": w_arr,
            "ident": ident,
        })
    return in_maps, order, deg, Tw, groups, C_total


TRACE = False          # set by test harness to capture an NTFF profile
TRACE_CORES = None
LAST_RESULTS = None    # BassKernelResults of the most recent run


def kernel(query, key, attn_kernel, targets):
    global LAST_RESULTS
    query = np.asarray(query, dtype=np.float32)
    key = np.asarray(key, dtype=np.float32)
    attn_kernel = np.asarray(attn_kernel, dtype=np.float32)
    targets = np.asarray(targets, dtype=np.int32)

    _ensure_imports()
    from concourse.bass_utils import run_bass_kernel_spmd

    in_maps, order, deg, Tw, groups, C_total = _host_arrays(
        query, key, attn_kernel, targets)
    nc = build_nc(groups, Tw, C_total)
    res = run_bass_kernel_spmd(
        nc, in_maps, list(range(N_CORES)),
        trace=TRACE, trace_cores=TRACE_CORES,
    )
    LAST_RESULTS = res

    out = np.zeros((N_NODES, HC), dtype=np.float32)
    for cidx in range(N_CORES):
        r = np.asarray(res.results[cidx]["out"], dtype=np.float32)
        r = r.reshape(P, N_WIN, HC)
        nodes = order[cidx::N_CORES]
        l = np.arange(len(nodes), dtype=np.int64)
        out[nodes] = r[l % P, l // P, :]
    out[deg == 0] = 0.0
    return out


# revision 15
# speedup vs baseline: 1.0687x; 1.0687x over previous
"""GATv2 attention-pool kernel for 8 Trainium2 NeuronCores.

v2 "diagonal scatter" design
----------------------------
Reference computes, per edge e with target node t(e):
    feats = q + k                                   [E, 64]
    logits[e,h] = sum_c feats[e,h*8+c] * A[c,h]     [E, 8]
    attn = segment_softmax(logits, targets)         [E, 8]
    out[n] = relu(segment_sum(q * attn))            [N, 64]

Logits are O(10) so exp() never overflows fp32; the segment-max shift is
unnecessary and softmax folds into two segment-SUMS:
    denom[n,h]  = sum_{e->n} exp(logits[e,h])
    pooled[n,:] = sum_{e->n} q[e,:] * exp(logits[e,h])
    out[n]      = relu(pooled[n]) / denom[n]

Layout: nodes are sorted by degree and dealt round-robin to the 8 cores so
every core sees an identical degree profile (one SPMD program).  A window is
128 similar-degree nodes, ONE NODE PER SBUF PARTITION; each node's edge list
is padded to the window max Tw (~5% padding, pad rows are crafted so their
logits are ~-30 for every head -> exp ~ 1e-13 contributes nothing).  With
that layout the segment reductions are plain free-dim sums per partition:
    psum[128, 72] += I128^T @ m[:, t, 0:72]     (t = 0..Tw-1)
i.e. PSUM accumulation through the PE with a CONSTANT identity stationary
matrix - no per-subtile one-hot selector, no index tensors on device at all.
Everything ships and computes in bf16 (fp32 only for logits tail + PSUM).

Host work is index metadata + data layout only (degree sort, gather of q/k
rows into the padded slot order, bf16 cast); all floating-point math runs on
device.
"""

import os
import sys

import numpy as np

N_NODES = 100000
N_EDGES = 1600000
H = 8
C = 8
HC = H * C
N_CORES = 8
P = 128                       # nodes per window == SBUF partitions
NPC = N_NODES // N_CORES      # nodes per core
N_WIN = (NPC + P - 1) // P    # windows per core
TCAP = 72                     # max t-columns per processing group
NWCAP = 7                     # psum bank holds nw*72 fp32 <= 512
PAD_LG = -30.0                # logit forced onto pad slots
F_VIA_DMA_ACCUM = False       # build f=q+k with an accumulating SWDGE DMA


def _ensure_imports():
    try:
        import concourse.bass  # noqa: F401
    except ImportError:
        for p in ("/opt/trn_rl_repo", "/root/.axon_site/_ro/trn_rl_repo"):
            if os.path.isdir(p) and p not in sys.path:
                sys.path.insert(0, p)


def _run_arange(counts):
    """[0..c0-1, 0..c1-1, ...] for run lengths `counts`."""
    tot = int(counts.sum())
    a = np.arange(tot, dtype=np.int64)
    off = np.repeat(np.cumsum(counts) - counts, counts)
    return a - off


def preprocess(targets):
    """Degree-sort nodes, deal round-robin to cores, pack windows.

    Returns (order, deg, Tw [N_WIN], c0 [N_WIN+1], groups, C_total,
    idx_maps: per-core [P, C_total] int64 edge id or -1 for pad).
    """
    deg = np.bincount(targets, minlength=N_NODES).astype(np.int64)
    order = np.argsort(deg, kind="stable")          # ascending degree
    Tw = np.zeros(N_WIN, dtype=np.int64)
    for w in range(N_WIN):
        lo = w * P * N_CORES
        hi = min((w + 1) * P * N_CORES, N_NODES)
        Tw[w] = max(1, int(deg[order[lo:hi]].max()))
    c0 = np.zeros(N_WIN + 1, dtype=np.int64)
    c0[1:] = np.cumsum(Tw)
    C_total = int(c0[-1])

    groups = []  # (w0, nw, T, cstart)
    w = 0
    while w < N_WIN:
        w0 = w
        T = 0
        while w < N_WIN and (w - w0) < NWCAP and T + Tw[w] <= TCAP:
            T += int(Tw[w])
            w += 1
        if w == w0:          # single window wider than TCAP
            T = int(Tw[w])
            w += 1
        groups.append((w0, w - w0, T, int(c0[w0])))
    # big groups first: better pipeline ramp, small tail
    groups.sort(key=lambda g: -g[2])

    eorder = np.argsort(targets, kind="stable")
    tsorted = targets[eorder]
    estart = np.searchsorted(tsorted, np.arange(N_NODES + 1))

    idx_maps = []
    for cidx in range(N_CORES):
        nodes = order[cidx::N_CORES]                # local rank l -> node
        l = np.arange(len(nodes), dtype=np.int64)
        wloc = l // P
        ploc = l % P
        d = deg[nodes]
        dstart = ploc * C_total + c0[wloc]
        dst = np.repeat(dstart, d) + _run_arange(d)
        src = np.repeat(estart[nodes], d) + _run_arange(d)
        idx = np.full(P * C_total, -1, dtype=np.int64)
        idx[dst] = eorder[src]
        idx_maps.append(idx.reshape(P, C_total))
    return order, deg, Tw, c0, groups, C_total, idx_maps


def build_nc(groups, Tw, C_total):
    """Single SPMD Bass program for one core's shard."""
    _ensure_imports()
    import concourse.bacc as bacc
    import concourse.mybir as mybir
    import concourse.tile as tile

    f32 = mybir.dt.float32
    bf16 = mybir.dt.bfloat16
    f16 = mybir.dt.float16
    AF = mybir.ActivationFunctionType
    OP = mybir.AluOpType

    # fp16 for the small-range logits path (8x finer mantissa than bf16);
    # bf16 only where exp() range demands it (eb, m); fp32 logits + PSUM.
    nc = bacc.Bacc("TRN2", num_devices=N_CORES)
    qd = nc.declare_dram_parameter("q", [P, C_total * HC], f16, False)
    kd = nc.declare_dram_parameter("k", [P, C_total * HC], f16, False)
    wd = nc.declare_dram_parameter("w", [P, TCAP * HC], f16, False)
    idd = nc.declare_dram_parameter("ident", [P, P], bf16, False)
    outd = nc.declare_dram_parameter("out", [P, N_WIN * HC], f16,
                                     isOutput=True)

    with tile.TileContext(nc) as tc:
        with (
            tc.tile_pool(name="const", bufs=1) as cpool,
            tc.tile_pool(name="qin", bufs=4) as qpool,
            tc.tile_pool(name="kin", bufs=4) as kpool,
            tc.tile_pool(name="mid", bufs=2) as midpool,
            tc.tile_pool(name="mm", bufs=2) as mpool,
            tc.tile_pool(name="fin", bufs=3) as finpool,
            tc.tile_pool(name="psum", bufs=8, space="PSUM") as ppool,
        ):
            w_t = cpool.tile([P, TCAP * HC], f16)
            nc.sync.dma_start(out=w_t[:], in_=wd[:])
            id_t = cpool.tile([P, P], bf16)
            nc.sync.dma_start(out=id_t[:], in_=idd[:])

            for (w0, nw, T, cs) in groups:
                fd = T * HC
                qt = qpool.tile([P, fd], f16, tag="q")
                nc.sync.dma_start(out=qt[:], in_=qd[:, cs * HC:(cs + T) * HC])

                if F_VIA_DMA_ACCUM:
                    # f = q + k built by the DMA engines: load q a second
                    # time, then the SWDGE k-load accumulates on top.
                    ft = kpool.tile([P, fd], f16, tag="f")
                    nc.sync.dma_start(out=ft[:],
                                      in_=qd[:, cs * HC:(cs + T) * HC])
                    nc.gpsimd.dma_start(out=ft[:],
                                        in_=kd[:, cs * HC:(cs + T) * HC],
                                        accum_op=OP.add)
                else:
                    kt = kpool.tile([P, fd], f16, tag="k")
                    nc.sync.dma_start(out=kt[:],
                                      in_=kd[:, cs * HC:(cs + T) * HC])
                    ft = midpool.tile([P, fd], f16, tag="f")
                    nc.vector.tensor_add(ft[:], qt[:], kt[:])
                wf = midpool.tile([P, fd], f16, tag="wf")
                nc.vector.tensor_mul(wf[:], ft[:], w_t[:, :fd])

                # tree-reduce the 8 channels per (t, head)
                wf4 = wf[:].rearrange("p (x c) -> p x c", c=8)
                t1 = midpool.tile([P, T * 8, 4], f16, tag="t1")
                nc.vector.tensor_add(t1[:], wf4[:, :, 0:4], wf4[:, :, 4:8])
                t2 = midpool.tile([P, T * 8, 2], f16, tag="t2")
                nc.vector.tensor_add(t2[:], t1[:, :, 0:2], t1[:, :, 2:4])
                lg = midpool.tile([P, T * 8], f32, tag="lg")
                nc.vector.tensor_add(lg[:], t2[:, :, 0], t2[:, :, 1])

                lg3 = lg[:].rearrange("p (t h) -> p t h", h=H)
                eb = midpool.tile([P, fd], bf16, tag="eb")
                nc.scalar.activation(
                    out=eb[:].rearrange("p (t h c) -> p t h c", h=H, c=C),
                    in_=lg3[:, :, :, None].to_broadcast([P, T, H, C]),
                    func=AF.Exp,
                )

                m = mpool.tile([P, T, 72], bf16, tag="m")
                nc.vector.tensor_mul(
                    m[:, :, 0:HC],
                    qt[:].rearrange("p (t j) -> p t j", j=HC),
                    eb[:].rearrange("p (t j) -> p t j", j=HC))
                nc.scalar.activation(out=m[:, :, HC:72], in_=lg3, func=AF.Exp)

                ps = ppool.tile([P, nw * 72], f32)
                tg = 0
                for wi in range(nw):
                    tw = int(Tw[w0 + wi])
                    for t in range(tw):
                        nc.tensor.matmul(
                            ps[:, wi * 72:(wi + 1) * 72],
                            lhsT=id_t[:],
                            rhs=m[:, tg + t, :],
                            start=(t == 0),
                            stop=(t == tw - 1),
                        )
                    tg += tw

                ps3 = ps[:].rearrange("p (w j) -> p w j", j=72)
                o = finpool.tile([P, nw, HC], f32, tag="o")
                nc.scalar.activation(out=o[:], in_=ps3[:, :, 0:HC],
                                     func=AF.Relu)
                rc = finpool.tile([P, nw, H], f32, tag="rc")
                nc.vector.reciprocal_approx_fast(rc[:], ps3[:, :, HC:72])
                o2 = finpool.tile([P, nw, HC], f16, tag="o2")
                nc.vector.tensor_mul(
                    o2[:].rearrange("p w (h c) -> p w h c", h=H),
                    o[:].rearrange("p w (h c) -> p w h c", h=H),
                    rc[:, :, :, None].to_broadcast([P, nw, H, C]),
                )
                nc.sync.dma_start(
                    out=outd[:, w0 * HC:(w0 + nw) * HC],
                    in_=o2[:].rearrange("p w j -> p (w j)"),
                )

    nc.finalize()
    return nc


def _host_arrays(query, key, attn_kernel, targets):
    import ml_dtypes

    bf = ml_dtypes.bfloat16
    f16 = np.float16
    order, deg, Tw, c0, groups, C_total, idx_maps = preprocess(targets)

    qb = query.astype(f16)
    kb = key.astype(f16)

    # pad rows: q = 0; k chosen so logits[h] == PAD_LG for every head
    A = attn_kernel.astype(np.float64)               # [C, H]
    nrm = np.maximum((A * A).sum(axis=0), 1e-6)      # ||A[:,h]||^2
    v = (PAD_LG / nrm)[None, :] * A                  # [C, H]
    kpad_row = np.ascontiguousarray(v.T).reshape(-1).astype(f16)  # [h*8+c]

    wrow = np.ascontiguousarray(attn_kernel.T).reshape(-1)  # [h*8+c] = A[c,h]
    w_arr = np.tile(wrow, (P, TCAP)).astype(f16)
    ident = np.eye(P, dtype=np.float32).astype(bf)

    in_maps = []
    for cidx in range(N_CORES):
        idx = idx_maps[cidx]
        safe = np.maximum(idx, 0)
        pad = idx < 0
        qdev = qb[safe]
        qdev[pad] = 0
        kdev = kb[safe]
        kdev[pad] = kpad_row
        in_maps.append({
            "q": np.ascontiguousarray(qdev.reshape(P, C_total * HC)),
            "k": np.ascontiguousarray(kdev.reshape(P, C_total * HC)),
            "w": w_arr,
            "ident": ident,
        })
    return in_maps, order, deg, Tw, groups, C_total


TRACE = False          # set by test harness to capture an NTFF profile
TRACE_CORES = None
LAST_RESULTS = None    # BassKernelResults of the most recent run


def kernel(query, key, attn_kernel, targets):
    global LAST_RESULTS
    query = np.asarray(query, dtype=np.float32)
    key = np.asarray(key, dtype=np.float32)
    attn_kernel = np.asarray(attn_kernel, dtype=np.float32)
    targets = np.asarray(targets, dtype=np.int32)

    _ensure_imports()
    from concourse.bass_utils import run_bass_kernel_spmd

    in_maps, order, deg, Tw, groups, C_total = _host_arrays(
        query, key, attn_kernel, targets)
    nc = build_nc(groups, Tw, C_total)
    res = run_bass_kernel_spmd(
        nc, in_maps, list(range(N_CORES)),
        trace=TRACE, trace_cores=TRACE_CORES,
    )
    LAST_RESULTS = res

    out = np.zeros((N_NODES, HC), dtype=np.float32)
    for cidx in range(N_CORES):
        r = np.asarray(res.results[cidx]["out"], dtype=np.float32)
        r = r.reshape(P, N_WIN, HC)
        nodes = order[cidx::N_CORES]
        l = np.arange(len(nodes), dtype=np.int64)
        out[nodes] = r[l % P, l // P, :]
    out[deg == 0] = 0.0
    return out
